# revision 11
# baseline (speedup 1.0000x reference)
"""Deformable-conv Bass kernel for TRN2, 8-way data-parallel.

Problem (nn_Deform): x[4,256,64,64] f32 ->
  off  = clip(conv3x3_pad1(x, offset_w) + offset_b, -1, 1)   # 18 offset planes
  cols = bilinear_sample(x, off)                             # 9 taps, dil=2 pad=2
  out  = einsum('bcnhw,ocn->bohw', cols, deform_w) + deform_b
returns (out, deform_w)

Formulation used on device (validated vs reference in fp32/fp16):
  Since off in [-1,1], each tap's bilinear sample is exactly
     cols_n = sum_{dy,dx in {-1,0,1}} wy_dy*wx_dx * x_shifted(base_n+(dy,dx))
  with hat weights wy_{-1}=relu(-t), wy_0=1-|t|, wy_{+1}=relu(t), t=off_y
  (same for x). Out-of-image contributions vanish via a 3-wide zero halo.

Sharding: 8 cores = (batch 4) x (H halves 2). Each core computes
out[b, :, h0:h0+32, :] from a host-prepared padded fp16 slice of x.

Device pipeline per core:
  PE : offset conv (K=c, M=9 per dy/dx group), weight-field partition
       broadcasts (one-hot selector matmuls), main GEMM (fp16, fp32 psum)
  ACT: hat construction, psum->sbuf evictions/casts
  DVE: clips, 9-shift multiply-accumulate into cols (fp16)
"""

import numpy as np

import concourse.bass as bass
import concourse.bacc as bacc
import concourse.mybir as mybir
from concourse import tile
from concourse.bass_utils import run_bass_kernel_spmd

F32 = mybir.dt.float32
F16 = mybir.dt.float16
Alu = mybir.AluOpType
ACT = mybir.ActivationFunctionType

B, C, H, W, O = 4, 256, 64, 64, 256
NCORES = 8
SH = H // 2          # rows per core shard
HALO = 3             # max |base + local shift|
ROWS = SH + 2 * HALO  # 38 padded rows per shard
COLS = W + 2 * HALO   # 70 padded cols
PIX = SH * W          # 2048 output pixels per core
TS = 1024             # pixel tile (16 output rows)
NTILES = PIX // TS
GP_TAPS = {2, 5, 8}       # taps whose MAC chain runs on GpSimd

_NC_CACHE = {}


def _taps():
    out = []
    for n in range(9):
        kh, kw = n // 3, n % 3
        out.append((-2 + 2 * kh, -2 + 2 * kw))  # (by, bx), PAD=2 DIL=2
    return out


def _build_nc():
    nc = bacc.Bacc(None, target_bir_lowering=False)
    x_d = nc.dram_tensor("xpad", [128, 2 * ROWS * COLS], F16, kind="ExternalInput")
    ow_d = nc.dram_tensor("offw", [128, 2 * 9 * 18], F16, kind="ExternalInput")
    dw_d = nc.dram_tensor("dwT", [128, 2 * 9 * O], F16, kind="ExternalInput")
    es_d = nc.dram_tensor("esel", [9, 9 * 128], F16, kind="ExternalInput")
    ob_d = nc.dram_tensor("offb", [9, 2], F32, kind="ExternalInput")
    db_d = nc.dram_tensor("dbias", [128, 2], F32, kind="ExternalInput")
    out_d = nc.dram_tensor("out", [O, PIX], F32, kind="ExternalOutput")

    taps = _taps()

    with tile.TileContext(nc) as tc:
        with tc.tile_pool(name="const", bufs=1) as cp:
            x16 = cp.tile([128, 2, ROWS, COLS], F16)
            offw = cp.tile([128, 2, 9, 18], F16)
            dwT = cp.tile([128, 2, 9, O], F16)
            esel = cp.tile([9, 9, 128], F16)
            offb = cp.tile([9, 2], F32)
            db = cp.tile([128, 2], F32)
            off2 = cp.tile([9, 2, PIX], F32)
            abs9 = cp.tile([9, 2, PIX], F32)
            H9 = cp.tile([9, 2, 3, PIX], F16)
            W81 = cp.tile([9, 9, PIX], F16)

            nc.sync.dma_start(out=x16[:], in_=x_d[:].rearrange(
                "p (b r c) -> p b r c", b=2, r=ROWS))
            nc.sync.dma_start(out=offw[:], in_=ow_d[:].rearrange(
                "p (b n o) -> p b n o", b=2, n=9))
            nc.sync.dma_start(out=dwT[:], in_=dw_d[:].rearrange(
                "p (b n o) -> p b n o", b=2, n=9))
            nc.sync.dma_start(out=esel[:], in_=es_d[:].rearrange(
                "p (n m) -> p n m", n=9))
            nc.sync.dma_start(out=offb[:], in_=ob_d[:])
            nc.sync.dma_start(out=db[:], in_=db_d[:])

            # Pre-touch DMA'd tiles on DVE: TensorScalarPtr instructions can
            # hold only one sync wait, so let plain copies absorb the DMA
            # semaphores before any tensor_scalar consumer runs.
            biasP = cp.tile([9, 2], F32)
            bias2 = cp.tile([9, 1], F32)
            db_s = cp.tile([128, 2], F32)
            touch = cp.tile([128, 2, 1], F16)
            nc.vector.memset(bias2[:], 2.0)
            nc.vector.tensor_scalar(out=biasP[:], in0=offb[:], scalar1=1.0,
                                    scalar2=None, op0=Alu.add)
            nc.vector.tensor_copy(db_s[:], db[:])
            nc.vector.tensor_copy(touch[:], x16[:, :, 0, 0:1])

            # ---- Stage 1: offset conv (3x3, pad 1, dil 1) -> off2[9, 2, PIX]
            with tc.tile_pool(name="pconv", bufs=1, space="PSUM") as pc:
                pdy = pc.tile([9, PIX], F32, tag="pdy")
                pdx = pc.tile([9, PIX], F32, tag="pdx")
                for s4 in range(4):  # 512-pixel (8-row) slabs
                    for cb in range(2):
                        for t9 in range(9):
                            ki, kj = t9 // 3, t9 % 3
                            rhs = x16[:, cb,
                                      HALO + (ki - 1) + 8 * s4: HALO + (ki - 1) + 8 * s4 + 8,
                                      HALO + (kj - 1): HALO + (kj - 1) + W]
                            first = (cb == 0 and t9 == 0)
                            last = (cb == 1 and t9 == 8)
                            nc.tensor.matmul(pdy[:, 512 * s4: 512 * (s4 + 1)],
                                             lhsT=offw[:, cb, t9, 0:9], rhs=rhs,
                                             start=first, stop=last)
                            nc.tensor.matmul(pdx[:, 512 * s4: 512 * (s4 + 1)],
                                             lhsT=offw[:, cb, t9, 9:18], rhs=rhs,
                                             start=first, stop=last)
                for g, pt in ((0, pdy), (1, pdx)):
                    # clip(v + b, -1, 1) = 1 - relu(2 - relu(v + b + 1)),
                    # done on ACT (TensorScalarPtr can hold only one sync wait,
                    # and this read of PSUM needs two).
                    nc.scalar.activation(off2[:, g, :], pt[:], ACT.Relu,
                                         bias=biasP[:, g:g + 1], scale=1.0)
                    nc.scalar.activation(off2[:, g, :], off2[:, g, :], ACT.Relu,
                                         bias=bias2[:], scale=-1.0)
                    nc.scalar.activation(off2[:, g, :], off2[:, g, :], ACT.Copy,
                                         bias=1.0, scale=-1.0)

            # ---- Stage 2: hat weights H9[n, g, {-1,0,+1}, pix]
            nc.scalar.activation(H9[:, :, 0, :], off2[:], ACT.Relu, scale=-1.0)
            nc.scalar.activation(H9[:, :, 2, :], off2[:], ACT.Relu, scale=1.0)
            nc.scalar.activation(abs9[:], off2[:], ACT.Abs)
            nc.vector.tensor_scalar(out=H9[:, :, 1, :], in0=abs9[:],
                                    scalar1=-1.0, scalar2=1.0,
                                    op0=Alu.mult, op1=Alu.add)

            # ---- Stage 3: W81[n, jk, pix] = wy_j(n) * wx_k(n)
            hy = H9[:, 0, :, :]   # [9, 3, PIX]
            hx = H9[:, 1, :, :]
            in0 = bass.AP(hy.tensor, hy.offset,
                          [hy.ap[0], hy.ap[1], [0, 3], hy.ap[2]])
            in1 = bass.AP(hx.tensor, hx.offset,
                          [hx.ap[0], [0, 3], hx.ap[1], hx.ap[2]])
            w81v = W81[:].rearrange("p (j k) x -> p j k x", j=3)
            nc.vector.tensor_tensor(out=w81v, in0=in0, in1=in1, op=Alu.mult)

            # ---- Stage 4: per pixel tile, per tap: broadcast W, MAC, GEMM
            with tc.tile_pool(name="wb", bufs=2) as wbp, \
                 tc.tile_pool(name="pbc", bufs=2, space="PSUM") as pbc, \
                 tc.tile_pool(name="colsp", bufs=2) as colsp, \
                 tc.tile_pool(name="tmpp", bufs=2) as tmpp, \
                 tc.tile_pool(name="pout", bufs=1, space="PSUM") as poutp, \
                 tc.tile_pool(name="outp", bufs=2) as outp:
                for t in range(NTILES):
                    pot = poutp.tile([128, 2, TS], F32, tag="pot")
                    for n in range(9):
                        by, bx = taps[n]
                        # broadcast the 9 shift-weight fields of tap n to 128 partitions
                        wbt = wbp.tile([128, 9, TS], F16, tag="wbt")
                        for jk in range(9):
                            pb = pbc.tile([128, TS], F32, tag="pb")
                            for s in range(TS // 512):
                                nc.tensor.matmul(
                                    pb[:, 512 * s: 512 * (s + 1)], lhsT=esel[:, n, :],
                                    rhs=W81[:, jk, t * TS + 512 * s: t * TS + 512 * (s + 1)],
                                    start=True, stop=True)
                            nc.scalar.activation(wbt[:, jk, :], pb[:], ACT.Copy)
                        # 9-shift MAC into cols (fp16); ~1/3 of taps run their
                        # chain on GpSimd to offload the DVE bottleneck
                        eng = nc.gpsimd if n in GP_TAPS else nc.vector
                        cols = colsp.tile([128, 2, TS], F16, tag="cols")
                        colsv = cols[:].rearrange("p b (r c) -> p b r c", r=TS // W)
                        for jk in range(9):
                            dyl, dxl = jk // 3 - 1, jk % 3 - 1
                            r0 = HALO + by + dyl + (TS // W) * t
                            c0 = HALO + bx + dxl
                            xs = x16[:, :, r0: r0 + TS // W, c0: c0 + W]
                            wj = wbt[:, jk, :]
                            wv = bass.AP(wj.tensor, wj.offset,
                                         [wj.ap[0], [0, 2], [W, TS // W], [1, W]])
                            if jk == 0:
                                eng.tensor_tensor(out=colsv, in0=xs, in1=wv,
                                                  op=Alu.mult)
                            else:
                                tmp = tmpp.tile([128, 2, TS], F16, tag="tmp")
                                tmpv = tmp[:].rearrange("p b (r c) -> p b r c",
                                                        r=TS // W)
                                eng.tensor_tensor(out=tmpv, in0=xs, in1=wv,
                                                  op=Alu.mult)
                                eng.tensor_tensor(out=cols[:], in0=cols[:],
                                                  in1=tmp[:], op=Alu.add)
                        # GEMM: accumulate this tap into out psum
                        for cb in range(2):
                            for ob in range(2):
                                for s in range(TS // 512):
                                    nc.tensor.matmul(
                                        pot[:, ob, 512 * s: 512 * (s + 1)],
                                        lhsT=dwT[:, cb, n, ob * 128:(ob + 1) * 128],
                                        rhs=cols[:, cb, 512 * s: 512 * (s + 1)],
                                        start=(n == 0 and cb == 0),
                                        stop=(n == 8 and cb == 1))
                    for ob in range(2):
                        outs = outp.tile([128, TS], F32, tag="outs")
                        # psum + per-o bias on ACT (multi-wait capable)
                        nc.scalar.activation(outs[:], pot[:, ob, :], ACT.Identity,
                                             bias=db_s[:, ob:ob + 1], scale=1.0)
                        nc.sync.dma_start(
                            out=out_d[ob * 128:(ob + 1) * 128, t * TS:(t + 1) * TS],
                            in_=outs[:])
    nc.finalize()
    return nc


def _host_prep(x, offset_w, offset_b, deform_w, deform_b):
    """Build the 8 per-core input maps (numpy, host-side layout prep only)."""
    x = np.asarray(x, np.float32)
    offset_w = np.asarray(offset_w, np.float32)
    offset_b = np.asarray(offset_b, np.float32)
    deform_w = np.asarray(deform_w, np.float32)
    deform_b = np.asarray(deform_b, np.float32)

    # offset_w channel perm: [dy_0..dy_8, dx_0..dx_8]
    perm = [2 * n for n in range(9)] + [2 * n + 1 for n in range(9)]
    oww = offset_w[perm]                                  # [18, C, 3, 3]
    # offw[p, cb, t9, o'] fp16
    offw = oww.reshape(18, 2, 128, 9).transpose(2, 1, 3, 0)
    offw = np.ascontiguousarray(offw, np.float16).reshape(128, -1)

    dwT = deform_w.reshape(O, 2, 128, 9).transpose(2, 1, 3, 0)
    dwT = np.ascontiguousarray(dwT, np.float16).reshape(128, -1)

    esel = np.zeros((9, 9, 128), np.float16)
    for n in range(9):
        esel[n, n, :] = 1.0
    esel = esel.reshape(9, -1)

    offb = np.ascontiguousarray(offset_b[perm].reshape(2, 9).T, np.float32)
    db = np.ascontiguousarray(deform_b.reshape(2, 128).T, np.float32)

    in_maps = []
    for core in range(NCORES):
        b, h0 = core // 2, SH * (core % 2)
        xs = np.zeros((C, ROWS, COLS), np.float32)
        lo, hi = max(0, h0 - HALO), min(H, h0 + SH + HALO)
        xs[:, lo - (h0 - HALO): hi - (h0 - HALO), HALO: HALO + W] = x[b, :, lo:hi, :]
        xs = xs.astype(np.float16).reshape(2, 128, ROWS, COLS).transpose(1, 0, 2, 3)
        in_maps.append({
            "xpad": np.ascontiguousarray(xs).reshape(128, -1),
            "offw": offw, "dwT": dwT, "esel": esel, "offb": offb, "dbias": db,
        })
    return in_maps


def kernel(x, offset_w, offset_b, deform_w, deform_b, _trace=False):
    in_maps = _host_prep(x, offset_w, offset_b, deform_w, deform_b)
    if "nc" not in _NC_CACHE:
        _NC_CACHE["nc"] = _build_nc()
    nc = _NC_CACHE["nc"]
    res = run_bass_kernel_spmd(nc, in_maps, core_ids=list(range(NCORES)),
                               trace=_trace)
    out = np.empty((B, O, H, W), np.float32)
    for core in range(NCORES):
        b, h0 = core // 2, SH * (core % 2)
        out[b, :, h0:h0 + SH, :] = res.results[core]["out"].reshape(O, SH, W)
    if _trace:
        kernel.last_exec_time_ns = res.exec_time_ns
        kernel.last_results = res
    return (out, np.asarray(deform_w, np.float32))


# revision 16
# speedup vs baseline: 1.3097x; 1.3097x over previous
"""Deformable-conv Bass kernel for TRN2, 8-way data-parallel.

Problem (nn_Deform): x[4,256,64,64] f32 ->
  off  = clip(conv3x3_pad1(x, offset_w) + offset_b, -1, 1)   # 18 offset planes
  cols = bilinear_sample(x, off)                             # 9 taps, dil=2 pad=2
  out  = einsum('bcnhw,ocn->bohw', cols, deform_w) + deform_b
returns (out, deform_w)

Formulation used on device (validated vs reference in fp32/fp16):
  Since off in [-1,1], each tap's bilinear sample is exactly
     cols_n = sum_{dy,dx in {-1,0,1}} wy_dy*wx_dx * x_shifted(base_n+(dy,dx))
  with hat weights wy_{-1}=relu(-t), wy_0=1-|t|, wy_{+1}=relu(t), t=off_y
  (same for x). Out-of-image contributions vanish via a 3-wide zero halo.

Sharding: 8 cores = (batch 4) x (H halves 2). Each core computes
out[b, :, h0:h0+32, :] from a host-prepared padded fp16 slice of x.

Device pipeline per core:
  PE : offset conv (K=c, M=9 per dy/dx group), weight-field partition
       broadcasts (one-hot selector matmuls), main GEMM (fp16, fp32 psum)
  ACT: hat construction, psum->sbuf evictions/casts
  DVE: clips, 9-shift multiply-accumulate into cols (fp16)
"""

import numpy as np

import concourse.bass as bass
import concourse.bacc as bacc
import concourse.mybir as mybir
from concourse import tile
from concourse.bass_utils import run_bass_kernel_spmd

F32 = mybir.dt.float32
F16 = mybir.dt.float16
Alu = mybir.AluOpType
ACT = mybir.ActivationFunctionType

B, C, H, W, O = 4, 256, 64, 64, 256
NCORES = 8
SH = H // 2          # rows per core shard
HALO = 3             # max |base + local shift|
ROWS = SH + 2 * HALO  # 38 padded rows per shard
COLS = W + 2 * HALO   # 70 padded cols
PIX = SH * W          # 2048 output pixels per core
TS = 1024             # pixel tile (16 output rows)
NTILES = PIX // TS
GP_TAPS = {2, 5, 8}       # taps whose MAC chain runs on GpSimd

_NC_CACHE = {}


def _taps():
    out = []
    for n in range(9):
        kh, kw = n // 3, n % 3
        out.append((-2 + 2 * kh, -2 + 2 * kw))  # (by, bx), PAD=2 DIL=2
    return out


def _build_nc():
    nc = bacc.Bacc(None, target_bir_lowering=False)
    x_d = nc.dram_tensor("xpad", [128, 2 * ROWS * COLS], F16, kind="ExternalInput")
    ow_d = nc.dram_tensor("offw", [128, 2 * 9 * 18], F16, kind="ExternalInput")
    dw_d = nc.dram_tensor("dwT", [128, 2 * 9 * O], F16, kind="ExternalInput")
    es_d = nc.dram_tensor("esel", [9, 9 * 128], F16, kind="ExternalInput")
    id_d = nc.dram_tensor("ident", [128, 128], F16, kind="ExternalInput")
    ob_d = nc.dram_tensor("offb", [9, 2], F32, kind="ExternalInput")
    db_d = nc.dram_tensor("dbias", [128, 2], F32, kind="ExternalInput")
    out_d = nc.dram_tensor("out", [O, PIX], F32, kind="ExternalOutput")

    taps = _taps()

    with tile.TileContext(nc) as tc:
        with tc.tile_pool(name="const", bufs=1) as cp:
            x16 = cp.tile([128, 2, ROWS, COLS], F16)
            offw = cp.tile([128, 2, 9, 18], F16)
            dwT = cp.tile([128, 2, 9, O], F16)
            esel = cp.tile([9, 9, 128], F16)
            offb = cp.tile([9, 2], F32)
            db = cp.tile([128, 2], F32)
            off2 = cp.tile([9, 2, PIX], F32)
            H9 = cp.tile([9, 2, 3, PIX], F16)
            W81 = cp.tile([9, 9, PIX], F16)
            ident = cp.tile([128, 128], F16)

            nc.sync.dma_start(out=x16[:], in_=x_d[:].rearrange(
                "p (b r c) -> p b r c", b=2, r=ROWS))
            nc.sync.dma_start(out=offw[:], in_=ow_d[:].rearrange(
                "p (b n o) -> p b n o", b=2, n=9))
            nc.sync.dma_start(out=dwT[:], in_=dw_d[:].rearrange(
                "p (b n o) -> p b n o", b=2, n=9))
            nc.sync.dma_start(out=esel[:], in_=es_d[:].rearrange(
                "p (n m) -> p n m", n=9))
            nc.sync.dma_start(out=ident[:], in_=id_d[:])
            nc.sync.dma_start(out=offb[:], in_=ob_d[:])
            nc.sync.dma_start(out=db[:], in_=db_d[:])

            # Pre-touch DMA'd tiles on DVE: TensorScalarPtr instructions can
            # hold only one sync wait, so let plain copies absorb the DMA
            # semaphores before any tensor_scalar consumer runs.
            biasP = cp.tile([9, 2], F32)
            bias2 = cp.tile([9, 1], F32)
            db_s = cp.tile([128, 2], F32)
            touch = cp.tile([128, 2, 1], F16)
            nc.vector.memset(bias2[:], 2.0)
            nc.vector.tensor_scalar(out=biasP[:], in0=offb[:], scalar1=1.0,
                                    scalar2=None, op0=Alu.add)
            nc.vector.tensor_copy(db_s[:], db[:])
            nc.vector.tensor_copy(touch[:], x16[:, :, 0, 0:1])

            # ---- Stage 1: offset conv (3x3, pad 1, dil 1) -> off2[9, 2, PIX]
            with tc.tile_pool(name="pconv", bufs=1, space="PSUM") as pc:
                pdy = pc.tile([9, PIX], F32, tag="pdy")
                pdx = pc.tile([9, PIX], F32, tag="pdx")
                for s4 in range(4):  # 512-pixel (8-row) slabs
                    for cb in range(2):
                        for t9 in range(9):
                            ki, kj = t9 // 3, t9 % 3
                            rhs = x16[:, cb,
                                      HALO + (ki - 1) + 8 * s4: HALO + (ki - 1) + 8 * s4 + 8,
                                      HALO + (kj - 1): HALO + (kj - 1) + W]
                            first = (cb == 0 and t9 == 0)
                            last = (cb == 1 and t9 == 8)
                            nc.tensor.matmul(pdy[:, 512 * s4: 512 * (s4 + 1)],
                                             lhsT=offw[:, cb, t9, 0:9], rhs=rhs,
                                             start=first, stop=last)
                            nc.tensor.matmul(pdx[:, 512 * s4: 512 * (s4 + 1)],
                                             lhsT=offw[:, cb, t9, 9:18], rhs=rhs,
                                             start=first, stop=last)
                for g, pt in ((0, pdy), (1, pdx)):
                    # clip(v + b, -1, 1) = 1 - relu(2 - relu(v + b + 1)),
                    # done on ACT (TensorScalarPtr can hold only one sync wait,
                    # and this read of PSUM needs two).
                    nc.scalar.activation(off2[:, g, :], pt[:], ACT.Relu,
                                         bias=biasP[:, g:g + 1], scale=1.0)
                    nc.scalar.activation(off2[:, g, :], off2[:, g, :], ACT.Relu,
                                         bias=bias2[:], scale=-1.0)
                    nc.scalar.activation(off2[:, g, :], off2[:, g, :], ACT.Copy,
                                         bias=1.0, scale=-1.0)

            # ---- Stage 2: hat weights H9[n, g, {-1,0,+1}, pix]
            # wy0 = 1 - |t| = 1 - (relu(t) + relu(-t))
            nc.scalar.activation(H9[:, :, 0, :], off2[:], ACT.Relu, scale=-1.0)
            nc.scalar.activation(H9[:, :, 2, :], off2[:], ACT.Relu, scale=1.0)
            nc.vector.tensor_tensor(out=H9[:, :, 1, :], in0=H9[:, :, 0, :],
                                    in1=H9[:, :, 2, :], op=Alu.add)
            nc.vector.tensor_scalar(out=H9[:, :, 1, :], in0=H9[:, :, 1, :],
                                    scalar1=-1.0, scalar2=1.0,
                                    op0=Alu.mult, op1=Alu.add)

            # ---- Stage 3: W81[n, jk, pix] = wy_j(n) * wx_k(n)
            hy = H9[:, 0, :, :]   # [9, 3, PIX]
            hx = H9[:, 1, :, :]
            in0 = bass.AP(hy.tensor, hy.offset,
                          [hy.ap[0], hy.ap[1], [0, 3], hy.ap[2]])
            in1 = bass.AP(hx.tensor, hx.offset,
                          [hx.ap[0], [0, 3], hx.ap[1], hx.ap[2]])
            w81v = W81[:].rearrange("p (j k) x -> p j k x", j=3)
            nc.vector.tensor_tensor(out=w81v, in0=in0, in1=in1, op=Alu.mult)

            # ---- Stage 4: per pixel tile, per tap: broadcast W, multiply on
            # DVE, accumulate the 9 shift-products on PE (identity matmuls
            # into PSUM), then GEMM the tap into the output psum.
            with tc.tile_pool(name="wb", bufs=2) as wbp, \
                 tc.tile_pool(name="pbc", bufs=1, space="PSUM") as pbc, \
                 tc.tile_pool(name="colsp", bufs=2) as colsp, \
                 tc.tile_pool(name="tmpp", bufs=10) as tmpp, \
                 tc.tile_pool(name="pcols", bufs=2, space="PSUM") as pcolsp, \
                 tc.tile_pool(name="pout", bufs=1, space="PSUM") as poutp, \
                 tc.tile_pool(name="outp", bufs=2) as outp:
                for t in range(NTILES):
                    pot = poutp.tile([128, 2, TS], F32, tag="pot")
                    for n in range(9):
                        by, bx = taps[n]
                        # broadcast the 9 shift-weight fields of tap n to 128 partitions
                        wbt = wbp.tile([128, 9, TS], F16, tag="wbt")
                        for jk in range(9):
                            pb = pbc.tile([128, TS], F32, tag="pb")
                            for s in range(TS // 512):
                                nc.tensor.matmul(
                                    pb[:, 512 * s: 512 * (s + 1)], lhsT=esel[:, n, :],
                                    rhs=W81[:, jk, t * TS + 512 * s: t * TS + 512 * (s + 1)],
                                    start=True, stop=True)
                            nc.scalar.activation(wbt[:, jk, :], pb[:], ACT.Copy)
                        # 9 shift-products on DVE (fp16, 2x mode)
                        prods = []
                        for jk in range(9):
                            dyl, dxl = jk // 3 - 1, jk % 3 - 1
                            r0 = HALO + by + dyl + (TS // W) * t
                            c0 = HALO + bx + dxl
                            xs = x16[:, :, r0: r0 + TS // W, c0: c0 + W]
                            wj = wbt[:, jk, :]
                            wv = bass.AP(wj.tensor, wj.offset,
                                         [wj.ap[0], [0, 2], [W, TS // W], [1, W]])
                            tmp = tmpp.tile([128, 2, TS], F16, tag="tmp")
                            tmpv = tmp[:].rearrange("p b (r c) -> p b r c",
                                                    r=TS // W)
                            nc.vector.tensor_tensor(out=tmpv, in0=xs, in1=wv,
                                                    op=Alu.mult)
                            prods.append(tmp)
                        # sum the 9 products in PSUM via identity matmuls
                        cols = colsp.tile([128, 2, TS], F16, tag="cols")
                        for cb in range(2):
                            for s in range(TS // 512):
                                pcs = pcolsp.tile([128, 512], F32, tag="pcs")
                                for jk in range(9):
                                    nc.tensor.matmul(
                                        pcs[:], lhsT=ident[:],
                                        rhs=prods[jk][:, cb, 512 * s: 512 * (s + 1)],
                                        start=(jk == 0), stop=(jk == 8))
                                nc.scalar.activation(
                                    cols[:, cb, 512 * s: 512 * (s + 1)], pcs[:],
                                    ACT.Copy)
                        # GEMM: accumulate this tap into out psum
                        for cb in range(2):
                            for ob in range(2):
                                for s in range(TS // 512):
                                    nc.tensor.matmul(
                                        pot[:, ob, 512 * s: 512 * (s + 1)],
                                        lhsT=dwT[:, cb, n, ob * 128:(ob + 1) * 128],
                                        rhs=cols[:, cb, 512 * s: 512 * (s + 1)],
                                        start=(n == 0 and cb == 0),
                                        stop=(n == 8 and cb == 1))
                    for ob in range(2):
                        outs = outp.tile([128, TS], F32, tag="outs")
                        # psum + per-o bias on ACT (multi-wait capable)
                        nc.scalar.activation(outs[:], pot[:, ob, :], ACT.Identity,
                                             bias=db_s[:, ob:ob + 1], scale=1.0)
                        nc.sync.dma_start(
                            out=out_d[ob * 128:(ob + 1) * 128, t * TS:(t + 1) * TS],
                            in_=outs[:])
    nc.finalize()
    return nc


def _host_prep(x, offset_w, offset_b, deform_w, deform_b):
    """Build the 8 per-core input maps (numpy, host-side layout prep only)."""
    x = np.asarray(x, np.float32)
    offset_w = np.asarray(offset_w, np.float32)
    offset_b = np.asarray(offset_b, np.float32)
    deform_w = np.asarray(deform_w, np.float32)
    deform_b = np.asarray(deform_b, np.float32)

    # offset_w channel perm: [dy_0..dy_8, dx_0..dx_8]
    perm = [2 * n for n in range(9)] + [2 * n + 1 for n in range(9)]
    oww = offset_w[perm]                                  # [18, C, 3, 3]
    # offw[p, cb, t9, o'] fp16
    offw = oww.reshape(18, 2, 128, 9).transpose(2, 1, 3, 0)
    offw = np.ascontiguousarray(offw, np.float16).reshape(128, -1)

    dwT = deform_w.reshape(O, 2, 128, 9).transpose(2, 1, 3, 0)
    dwT = np.ascontiguousarray(dwT, np.float16).reshape(128, -1)

    esel = np.zeros((9, 9, 128), np.float16)
    for n in range(9):
        esel[n, n, :] = 1.0
    esel = esel.reshape(9, -1)
    ident = np.eye(128, dtype=np.float16)

    offb = np.ascontiguousarray(offset_b[perm].reshape(2, 9).T, np.float32)
    db = np.ascontiguousarray(deform_b.reshape(2, 128).T, np.float32)

    in_maps = []
    for core in range(NCORES):
        b, h0 = core // 2, SH * (core % 2)
        xs = np.zeros((C, ROWS, COLS), np.float32)
        lo, hi = max(0, h0 - HALO), min(H, h0 + SH + HALO)
        xs[:, lo - (h0 - HALO): hi - (h0 - HALO), HALO: HALO + W] = x[b, :, lo:hi, :]
        xs = xs.astype(np.float16).reshape(2, 128, ROWS, COLS).transpose(1, 0, 2, 3)
        in_maps.append({
            "xpad": np.ascontiguousarray(xs).reshape(128, -1),
            "offw": offw, "dwT": dwT, "esel": esel, "ident": ident,
            "offb": offb, "dbias": db,
        })
    return in_maps


def kernel(x, offset_w, offset_b, deform_w, deform_b, _trace=False):
    in_maps = _host_prep(x, offset_w, offset_b, deform_w, deform_b)
    if "nc" not in _NC_CACHE:
        _NC_CACHE["nc"] = _build_nc()
    nc = _NC_CACHE["nc"]
    res = run_bass_kernel_spmd(nc, in_maps, core_ids=list(range(NCORES)),
                               trace=_trace)
    out = np.empty((B, O, H, W), np.float32)
    for core in range(NCORES):
        b, h0 = core // 2, SH * (core % 2)
        out[b, :, h0:h0 + SH, :] = res.results[core]["out"].reshape(O, SH, W)
    if _trace:
        kernel.last_exec_time_ns = res.exec_time_ns
        kernel.last_results = res
    return (out, np.asarray(deform_w, np.float32))


# revision 17
# speedup vs baseline: 1.6384x; 1.2509x over previous
"""Deformable-conv Bass kernel for TRN2, 8-way data-parallel.

Problem (nn_Deform): x[4,256,64,64] f32 ->
  off  = clip(conv3x3_pad1(x, offset_w) + offset_b, -1, 1)   # 18 offset planes
  cols = bilinear_sample(x, off)                             # 9 taps, dil=2 pad=2
  out  = einsum('bcnhw,ocn->bohw', cols, deform_w) + deform_b
returns (out, deform_w)

Formulation used on device (validated vs reference in fp32/fp16):
  Since off in [-1,1], each tap's bilinear sample is exactly
     cols_n = sum_{dy,dx in {-1,0,1}} wy_dy*wx_dx * x_shifted(base_n+(dy,dx))
  with hat weights wy_{-1}=relu(-t), wy_0=1-|t|, wy_{+1}=relu(t), t=off_y
  (same for x). Out-of-image contributions vanish via a 3-wide zero halo.

Sharding: 8 cores = (batch 4) x (H halves 2). Each core computes
out[b, :, h0:h0+32, :] from a host-prepared padded fp16 slice of x.

Device pipeline per core:
  PE : offset conv (K=c, M=9 per dy/dx group), weight-field partition
       broadcasts (one-hot selector matmuls), main GEMM (fp16, fp32 psum)
  ACT: hat construction, psum->sbuf evictions/casts
  DVE: clips, 9-shift multiply-accumulate into cols (fp16)
"""

import numpy as np

import concourse.bass as bass
import concourse.bacc as bacc
import concourse.mybir as mybir
from concourse import tile
from concourse.bass_utils import run_bass_kernel_spmd

F32 = mybir.dt.float32
F16 = mybir.dt.float16
Alu = mybir.AluOpType
ACT = mybir.ActivationFunctionType

B, C, H, W, O = 4, 256, 64, 64, 256
NCORES = 8
SH = H // 2          # rows per core shard
HALO = 3             # max |base + local shift|
ROWS = SH + 2 * HALO  # 38 padded rows per shard
COLS = W + 2 * HALO   # 70 padded cols
PIX = SH * W          # 2048 output pixels per core
TS = 1024             # pixel tile (16 output rows)
NTILES = PIX // TS
GP_TAPS = {2, 5, 8}       # taps whose MAC chain runs on GpSimd

_NC_CACHE = {}


def _taps():
    out = []
    for n in range(9):
        kh, kw = n // 3, n % 3
        out.append((-2 + 2 * kh, -2 + 2 * kw))  # (by, bx), PAD=2 DIL=2
    return out


def _build_nc():
    nc = bacc.Bacc(None, target_bir_lowering=False)
    x_d = nc.dram_tensor("xpad", [128, 2 * ROWS * COLS], F16, kind="ExternalInput")
    ow_d = nc.dram_tensor("offw", [128, 2 * 9 * 18], F16, kind="ExternalInput")
    dw_d = nc.dram_tensor("dwT", [128, 2 * 9 * O], F16, kind="ExternalInput")
    es_d = nc.dram_tensor("esel", [9, 9 * 128], F16, kind="ExternalInput")
    id_d = nc.dram_tensor("ident", [128, 128], F16, kind="ExternalInput")
    ob_d = nc.dram_tensor("offb", [9, 2], F32, kind="ExternalInput")
    db_d = nc.dram_tensor("dbias", [128, 2], F32, kind="ExternalInput")
    out_d = nc.dram_tensor("out", [O, PIX], F32, kind="ExternalOutput")

    taps = _taps()

    with tile.TileContext(nc) as tc:
        with tc.tile_pool(name="const", bufs=1) as cp:
            x16 = cp.tile([128, 2, ROWS, COLS], F16)
            offw = cp.tile([128, 2, 9, 18], F16)
            dwT = cp.tile([128, 2, 9, O], F16)
            esel = cp.tile([9, 9, 128], F16)
            offb = cp.tile([9, 2], F32)
            db = cp.tile([128, 2], F32)
            off2 = cp.tile([9, 2, PIX], F32)
            H9 = cp.tile([9, 2, 3, PIX], F16)
            W81 = cp.tile([9, 9, PIX], F16)
            ident = cp.tile([128, 128], F16)

            nc.sync.dma_start(out=x16[:], in_=x_d[:].rearrange(
                "p (b r c) -> p b r c", b=2, r=ROWS))
            nc.sync.dma_start(out=offw[:], in_=ow_d[:].rearrange(
                "p (b n o) -> p b n o", b=2, n=9))
            nc.sync.dma_start(out=dwT[:], in_=dw_d[:].rearrange(
                "p (b n o) -> p b n o", b=2, n=9))
            nc.sync.dma_start(out=esel[:], in_=es_d[:].rearrange(
                "p (n m) -> p n m", n=9))
            nc.sync.dma_start(out=ident[:], in_=id_d[:])
            nc.sync.dma_start(out=offb[:], in_=ob_d[:])
            nc.sync.dma_start(out=db[:], in_=db_d[:])

            # Pre-touch DMA'd tiles on DVE: TensorScalarPtr instructions can
            # hold only one sync wait, so let plain copies absorb the DMA
            # semaphores before any tensor_scalar consumer runs.
            biasP = cp.tile([9, 2], F32)
            bias2 = cp.tile([9, 1], F32)
            db_s = cp.tile([128, 2], F32)
            touch = cp.tile([128, 2, 1], F16)
            nc.vector.memset(bias2[:], 2.0)
            nc.vector.tensor_scalar(out=biasP[:], in0=offb[:], scalar1=1.0,
                                    scalar2=None, op0=Alu.add)
            nc.vector.tensor_copy(db_s[:], db[:])
            nc.vector.tensor_copy(touch[:], x16[:, :, 0, 0:1])

            # ---- Stage 1: offset conv (3x3, pad 1, dil 1) -> off2[9, 2, PIX]
            with tc.tile_pool(name="pconv", bufs=1, space="PSUM") as pc:
                pdy = pc.tile([9, PIX], F32, tag="pdy")
                pdx = pc.tile([9, PIX], F32, tag="pdx")
                for s4 in range(4):  # 512-pixel (8-row) slabs
                    for cb in range(2):
                        for t9 in range(9):
                            ki, kj = t9 // 3, t9 % 3
                            rhs = x16[:, cb,
                                      HALO + (ki - 1) + 8 * s4: HALO + (ki - 1) + 8 * s4 + 8,
                                      HALO + (kj - 1): HALO + (kj - 1) + W]
                            first = (cb == 0 and t9 == 0)
                            last = (cb == 1 and t9 == 8)
                            nc.tensor.matmul(pdy[:, 512 * s4: 512 * (s4 + 1)],
                                             lhsT=offw[:, cb, t9, 0:9], rhs=rhs,
                                             start=first, stop=last)
                            nc.tensor.matmul(pdx[:, 512 * s4: 512 * (s4 + 1)],
                                             lhsT=offw[:, cb, t9, 9:18], rhs=rhs,
                                             start=first, stop=last)
                for g, pt in ((0, pdy), (1, pdx)):
                    # clip(v + b, -1, 1) = 1 - relu(2 - relu(v + b + 1)),
                    # done on ACT (TensorScalarPtr can hold only one sync wait,
                    # and this read of PSUM needs two).
                    nc.scalar.activation(off2[:, g, :], pt[:], ACT.Relu,
                                         bias=biasP[:, g:g + 1], scale=1.0)
                    nc.scalar.activation(off2[:, g, :], off2[:, g, :], ACT.Relu,
                                         bias=bias2[:], scale=-1.0)
                    nc.scalar.activation(off2[:, g, :], off2[:, g, :], ACT.Copy,
                                         bias=1.0, scale=-1.0)

            # ---- Stage 2: hat weights H9[n, g, {-1,0,+1}, pix]
            # wy0 = 1 - |t| = 1 - (relu(t) + relu(-t))
            nc.scalar.activation(H9[:, :, 0, :], off2[:], ACT.Relu, scale=-1.0)
            nc.scalar.activation(H9[:, :, 2, :], off2[:], ACT.Relu, scale=1.0)
            nc.vector.tensor_tensor(out=H9[:, :, 1, :], in0=H9[:, :, 0, :],
                                    in1=H9[:, :, 2, :], op=Alu.add)
            nc.vector.tensor_scalar(out=H9[:, :, 1, :], in0=H9[:, :, 1, :],
                                    scalar1=-1.0, scalar2=1.0,
                                    op0=Alu.mult, op1=Alu.add)

            # ---- Stage 3: W81[n, jk, pix] = wy_j(n) * wx_k(n)
            hy = H9[:, 0, :, :]   # [9, 3, PIX]
            hx = H9[:, 1, :, :]
            in0 = bass.AP(hy.tensor, hy.offset,
                          [hy.ap[0], hy.ap[1], [0, 3], hy.ap[2]])
            in1 = bass.AP(hx.tensor, hx.offset,
                          [hx.ap[0], [0, 3], hx.ap[1], hx.ap[2]])
            w81v = W81[:].rearrange("p (j k) x -> p j k x", j=3)
            nc.vector.tensor_tensor(out=w81v, in0=in0, in1=in1, op=Alu.mult)

            # ---- Stage 4: per pixel tile, per tap: broadcast W, MAC on DVE,
            # then GEMM the tap into the output psum.
            with tc.tile_pool(name="wb", bufs=3) as wbp, \
                 tc.tile_pool(name="pbc", bufs=2, space="PSUM") as pbc, \
                 tc.tile_pool(name="colsp", bufs=2) as colsp, \
                 tc.tile_pool(name="tmpp", bufs=3) as tmpp, \
                 tc.tile_pool(name="pout", bufs=1, space="PSUM") as poutp, \
                 tc.tile_pool(name="outp", bufs=2) as outp:
                for t in range(NTILES):
                    pot = poutp.tile([128, 2, TS], F32, tag="pot")
                    for n in range(9):
                        by, bx = taps[n]
                        # broadcast the 9 shift-weight fields of tap n to 128 partitions
                        wbt = wbp.tile([128, 9, TS], F16, tag="wbt")
                        for jk in range(9):
                            pb = pbc.tile([128, TS], F32, tag="pb")
                            for s in range(TS // 512):
                                nc.tensor.matmul(
                                    pb[:, 512 * s: 512 * (s + 1)], lhsT=esel[:, n, :],
                                    rhs=W81[:, jk, t * TS + 512 * s: t * TS + 512 * (s + 1)],
                                    start=True, stop=True)
                            nc.scalar.activation(wbt[:, jk, :], pb[:], ACT.Copy)
                        # 9-shift MAC into cols (fp16, DVE)
                        cols = colsp.tile([128, 2, TS], F16, tag="cols")
                        colsv = cols[:].rearrange("p b (r c) -> p b r c", r=TS // W)
                        for jk in range(9):
                            dyl, dxl = jk // 3 - 1, jk % 3 - 1
                            r0 = HALO + by + dyl + (TS // W) * t
                            c0 = HALO + bx + dxl
                            xs = x16[:, :, r0: r0 + TS // W, c0: c0 + W]
                            wj = wbt[:, jk, :]
                            wv = bass.AP(wj.tensor, wj.offset,
                                         [wj.ap[0], [0, 2], [W, TS // W], [1, W]])
                            if jk == 0:
                                nc.vector.tensor_tensor(out=colsv, in0=xs, in1=wv,
                                                        op=Alu.mult)
                            else:
                                tmp = tmpp.tile([128, 2, TS], F16, tag="tmp")
                                tmpv = tmp[:].rearrange("p b (r c) -> p b r c",
                                                        r=TS // W)
                                nc.vector.tensor_tensor(out=tmpv, in0=xs, in1=wv,
                                                        op=Alu.mult)
                                nc.vector.tensor_tensor(out=cols[:], in0=cols[:],
                                                        in1=tmp[:], op=Alu.add)
                        # GEMM: accumulate this tap into out psum
                        for cb in range(2):
                            for ob in range(2):
                                for s in range(TS // 512):
                                    nc.tensor.matmul(
                                        pot[:, ob, 512 * s: 512 * (s + 1)],
                                        lhsT=dwT[:, cb, n, ob * 128:(ob + 1) * 128],
                                        rhs=cols[:, cb, 512 * s: 512 * (s + 1)],
                                        start=(n == 0 and cb == 0),
                                        stop=(n == 8 and cb == 1))
                    for ob in range(2):
                        outs = outp.tile([128, TS], F32, tag="outs")
                        # psum + per-o bias on ACT (multi-wait capable)
                        nc.scalar.activation(outs[:], pot[:, ob, :], ACT.Identity,
                                             bias=db_s[:, ob:ob + 1], scale=1.0)
                        nc.sync.dma_start(
                            out=out_d[ob * 128:(ob + 1) * 128, t * TS:(t + 1) * TS],
                            in_=outs[:])
    nc.finalize()
    return nc


def _host_prep(x, offset_w, offset_b, deform_w, deform_b):
    """Build the 8 per-core input maps (numpy, host-side layout prep only)."""
    x = np.asarray(x, np.float32)
    offset_w = np.asarray(offset_w, np.float32)
    offset_b = np.asarray(offset_b, np.float32)
    deform_w = np.asarray(deform_w, np.float32)
    deform_b = np.asarray(deform_b, np.float32)

    # offset_w channel perm: [dy_0..dy_8, dx_0..dx_8]
    perm = [2 * n for n in range(9)] + [2 * n + 1 for n in range(9)]
    oww = offset_w[perm]                                  # [18, C, 3, 3]
    # offw[p, cb, t9, o'] fp16
    offw = oww.reshape(18, 2, 128, 9).transpose(2, 1, 3, 0)
    offw = np.ascontiguousarray(offw, np.float16).reshape(128, -1)

    dwT = deform_w.reshape(O, 2, 128, 9).transpose(2, 1, 3, 0)
    dwT = np.ascontiguousarray(dwT, np.float16).reshape(128, -1)

    esel = np.zeros((9, 9, 128), np.float16)
    for n in range(9):
        esel[n, n, :] = 1.0
    esel = esel.reshape(9, -1)
    ident = np.eye(128, dtype=np.float16)

    offb = np.ascontiguousarray(offset_b[perm].reshape(2, 9).T, np.float32)
    db = np.ascontiguousarray(deform_b.reshape(2, 128).T, np.float32)

    in_maps = []
    for core in range(NCORES):
        b, h0 = core // 2, SH * (core % 2)
        xs = np.zeros((C, ROWS, COLS), np.float32)
        lo, hi = max(0, h0 - HALO), min(H, h0 + SH + HALO)
        xs[:, lo - (h0 - HALO): hi - (h0 - HALO), HALO: HALO + W] = x[b, :, lo:hi, :]
        xs = xs.astype(np.float16).reshape(2, 128, ROWS, COLS).transpose(1, 0, 2, 3)
        in_maps.append({
            "xpad": np.ascontiguousarray(xs).reshape(128, -1),
            "offw": offw, "dwT": dwT, "esel": esel, "ident": ident,
            "offb": offb, "dbias": db,
        })
    return in_maps


def kernel(x, offset_w, offset_b, deform_w, deform_b, _trace=False):
    in_maps = _host_prep(x, offset_w, offset_b, deform_w, deform_b)
    if "nc" not in _NC_CACHE:
        _NC_CACHE["nc"] = _build_nc()
    nc = _NC_CACHE["nc"]
    res = run_bass_kernel_spmd(nc, in_maps, core_ids=list(range(NCORES)),
                               trace=_trace)
    out = np.empty((B, O, H, W), np.float32)
    for core in range(NCORES):
        b, h0 = core // 2, SH * (core % 2)
        out[b, :, h0:h0 + SH, :] = res.results[core]["out"].reshape(O, SH, W)
    if _trace:
        kernel.last_exec_time_ns = res.exec_time_ns
        kernel.last_results = res
    return (out, np.asarray(deform_w, np.float32))
